# revision 13
# baseline (speedup 1.0000x reference)
"""MeanStdFilter kernel for 8 Trainium2 NeuronCores.

Semantics (matches the sequential-Welford reference with M=0, S=S_in, n=0):
    S1[f] = sum_b x[b, f]            (global, over all 32768 rows)
    S2[f] = sum_b x[b, f]^2
    mean  = S1 / N
    M2    = S2 - S1^2 / N + S_in     (Welford M2 started from buffer S)
    var   = M2 / (N - 1)             (N = 32768 > 1)
    out   = (x - mean) / (sqrt(var) + 1e-5)
The input running-mean buffer M is overwritten by the first Welford step in
the reference, so it never affects the output.

Distribution: x is sharded 4096 rows per core. Each core keeps its shard
resident in SBUF (4 contiguous chunks of 8 row-tiles), computes partial raw
sums, AllReduces 8 KB of stats, finalizes redundantly on every core in a
packed [128,8] layout, then normalizes IN PLACE and stores. HBM traffic per
core = one 16.8 MB read + one 16.8 MB write.

Engine balance (HW-measured):
  - fp32 matmul streams ~2.4 ns/col -> ones-matmul 2.46+ us per [128,1024]
    tile; DVE fp32 tensor_tensor 1.22 us per tile. S1 split: 19 tiles on
    PE, 13 on DVE (acc1 chain), merged into the PSUM group at the end.
  - Concurrent GpSimd tensor_tensor degrades DVE 1.22 -> 3.3 us (shared
    SBUF port mux): normalize runs on DVE only, as 8 chunked ops of
    FD=8192 (amortizes the 151-cycle DVE instruction overhead).
  - Warmup AllReduce at kernel start primes CC rings / absorbs start skew.
"""

import functools

import numpy as np

import concourse.bacc as bacc
import concourse.tile as tile
from concourse import mybir
from concourse.bass_utils import run_bass_kernel_spmd

NCORES = 8
B, F = 32768, 1024
ROWS = B // NCORES  # 4096 rows per core
P = 128
NT = ROWS // P  # 32 row-tiles of [128, 1024] per core
CHUNKS = (10, 10, 8, 4)  # tiles per resident chunk (small tail chunk)
NCHUNK = len(CHUNKS)
CHUNK_OF = []
SLOT_OF = []
for _c, _n in enumerate(CHUNKS):
    CHUNK_OF += [_c] * _n
    SLOT_OF += list(range(_n))
EPS = 1e-5
FP32 = mybir.dt.float32
AF = mybir.ActivationFunctionType
ALU = mybir.AluOpType

# Tiles whose S1 contribution is accumulated on DVE instead of PE (13 of 32).
DVE_S1_TILES = frozenset(t for t in range(NT) if t % 5 in (1, 3))


def build_kernel():
    nc = bacc.Bacc(
        "TRN2", target_bir_lowering=False, debug=False, num_devices=NCORES
    )
    x = nc.declare_dram_parameter("x", [ROWS, F], FP32, isOutput=False)
    s_in = nc.declare_dram_parameter("S", [1, F], FP32, isOutput=False)
    out = nc.declare_dram_parameter("out", [ROWS, F], FP32, isOutput=True)

    x_t = x[:].rearrange("(n p) f -> n p f", p=P)
    out_t = out[:].rearrange("(n p) f -> n p f", p=P)
    groups = [list(range(NCORES))]

    with tile.TileContext(nc) as tc:
        with (
            tc.tile_pool(name="xbuf", bufs=1) as xpool,
            tc.tile_pool(name="work", bufs=3) as work,
            tc.tile_pool(name="stats", bufs=1) as stats,
            tc.tile_pool(name="psum", bufs=1, space="PSUM") as psum,
            tc.tile_pool(name="dram", bufs=1, space="DRAM") as dram,
        ):
            # Warmup AllReduce: primes the CC rings and synchronizes core
            # start skew while the load phase runs. Result is unused.
            wu = stats.tile([1, 8], FP32)
            nc.vector.memset(wu, 0.0)
            wu_in = dram.tile([1, 8], FP32)
            wu_out = dram.tile([1, 8], FP32)
            nc.gpsimd.dma_start(out=wu_in[:], in_=wu[:])
            nc.gpsimd.collective_compute(
                "AllReduce",
                ALU.add,
                replica_groups=groups,
                ins=[wu_in[:].opt()],
                outs=[wu_out[:].opt()],
            )

            ones = stats.tile([P, 1], FP32)
            nc.vector.memset(ones, 1.0)
            accsq = stats.tile([P, F], FP32)
            acc1 = stats.tile([P, F], FP32)

            # Resident shard: 4 chunks x [128, 8, 1024] (32 KB/partition each).
            xb = [
                xpool.tile([P, CHUNKS[c], F], FP32, tag=f"xb{c}", name=f"xb{c}")
                for c in range(NCHUNK)
            ]

            def xtile(t):
                return xb[CHUNK_OF[t]][:, SLOT_OF[t], :]

            # One PSUM bank per 512-wide half (fp32 matmul N<=512/bank).
            ps1 = [psum.tile([1, 512], FP32, tag=f"ps1_{h}", name=f"ps1_{h}") for h in range(2)]
            ps2 = [psum.tile([1, 512], FP32, tag=f"ps2_{h}", name=f"ps2_{h}") for h in range(2)]

            # ---- Phase A: load shard, accumulate raw sums ----
            first_dve = min(DVE_S1_TILES)
            first_pe = min(t for t in range(NT) if t not in DVE_S1_TILES)
            for t in range(NT):
                xt = xtile(t)
                nc.sync.dma_start(out=xt, in_=x_t[t])
                if t in DVE_S1_TILES:
                    if t == first_dve:
                        nc.vector.tensor_copy(acc1[:], xt)
                    else:
                        nc.vector.tensor_tensor(acc1[:], acc1, xt, ALU.add)
                else:
                    for h in range(2):
                        nc.tensor.matmul(
                            ps1[h][:],
                            lhsT=ones[:],
                            rhs=xt[:, h * 512 : (h + 1) * 512],
                            start=(t == first_pe),
                            stop=False,
                        )
                sq = work.tile([P, F], FP32, tag="sq")
                nc.scalar.activation(sq, xt, AF.Square)
                if t == 0:
                    nc.vector.tensor_copy(accsq[:], sq)
                else:
                    nc.vector.tensor_tensor(accsq[:], accsq, sq, ALU.add)

            # Merge the DVE-side S1 partial into the PSUM accumulation group,
            # and reduce accsq across partitions.
            for h in range(2):
                nc.tensor.matmul(
                    ps1[h][:],
                    lhsT=ones[:],
                    rhs=acc1[:, h * 512 : (h + 1) * 512],
                    start=False,
                    stop=True,
                )
                nc.tensor.matmul(
                    ps2[h][:],
                    lhsT=ones[:],
                    rhs=accsq[:, h * 512 : (h + 1) * 512],
                    start=True,
                    stop=True,
                )

            cc_stage = stats.tile([1, 2 * F], FP32)
            for h in range(2):
                nc.scalar.copy(cc_stage[:, h * 512 : (h + 1) * 512], ps1[h][:])
                nc.scalar.copy(
                    cc_stage[:, F + h * 512 : F + (h + 1) * 512], ps2[h][:]
                )

            cc_in = dram.tile([1, 2 * F], FP32)
            cc_out = dram.tile([1, 2 * F], FP32)
            nc.sync.dma_start(out=cc_in[:], in_=cc_stage[:])
            nc.gpsimd.collective_compute(
                "AllReduce",
                ALU.add,
                replica_groups=groups,
                ins=[cc_in[:].opt()],
                outs=[cc_out[:].opt()],
            )

            # ---- Packed finalize: [128, 8] per-feature layout (f = p*8+j).
            # All FD-8 ops, so the whole chain is ~2us instead of ~20us.
            # mean_b comes straight off a broadcast of the global S1 (one
            # DMA + one ACT scale), so phase-C subtracts can start ~6us
            # before rstd finishes its packed-finalize round trip.
            gs1b = stats.tile([P, F], FP32)
            nc.sync.dma_start(out=gs1b[:], in_=cc_out[:, 0:F].to_broadcast([P, F]))
            mean_b = stats.tile([P, F], FP32)
            nc.scalar.activation(mean_b, gs1b, AF.Copy, scale=1.0 / B)

            s12p = stats.tile([P, 2, 8], FP32)
            nc.sync.dma_start(
                out=s12p[:],
                in_=cc_out[:].rearrange("a (h p j) -> a p h j", h=2, p=P, j=8),
            )
            sinp = stats.tile([P, 8], FP32)
            nc.sync.dma_start(
                out=sinp[:], in_=s_in[:].rearrange("a (p j) -> a p j", p=P, j=8)
            )

            s1v = s12p[:, 0, :]
            s2v = s12p[:, 1, :]
            mr = stats.tile([P, 16], FP32)  # cols 0:8 mean, 8:16 rstd
            finw = stats.tile([P, 32], FP32)
            w1, w2, w3, w4 = (finw[:, 8 * i : 8 * (i + 1)] for i in range(4))
            nc.scalar.activation(mr[:, 0:8], s1v, AF.Copy, scale=1.0 / B)
            nc.vector.tensor_tensor(w1, s1v, mr[:, 0:8], ALU.mult)  # S1^2/N
            nc.vector.tensor_tensor(w2, s2v, w1, ALU.subtract)  # M2
            nc.vector.tensor_tensor(w2, w2, sinp[:], ALU.add)  # + S_in
            nc.scalar.activation(w3, w2, AF.Sqrt, scale=1.0 / (B - 1))  # std
            nc.scalar.activation(w4, w3, AF.Copy, bias=EPS)  # std + eps
            nc.vector.reciprocal(mr[:, 8:16], w4)

            # Round-trip through DRAM to broadcast per-feature mean/rstd
            # across all 128 partitions ([128,16] row-major == feature order).
            mr_d = dram.tile([1, 2 * F], FP32)
            nc.sync.dma_start(
                out=mr_d[:].rearrange("a (h p j) -> a p h j", h=2, p=P, j=8),
                in_=mr[:].rearrange("p (h j) -> p h j", h=2, j=8),
            )
            rstd_b = stats.tile([P, F], FP32)
            nc.sync.dma_start(
                out=rstd_b[:], in_=mr_d[:, F : 2 * F].to_broadcast([P, F])
            )

            # ---- Phase C: normalize in place, chunked (FD=8192 per op) ----
            tbase = 0
            for c in range(NCHUNK):
                k = CHUNKS[c]
                mb = mean_b[:, None, :].to_broadcast([P, k, F])
                rb = rstd_b[:, None, :].to_broadcast([P, k, F])
                nc.vector.tensor_tensor(xb[c][:], xb[c], mb, ALU.subtract)
                nc.vector.tensor_tensor(xb[c][:], xb[c], rb, ALU.mult)
                for j in range(k):
                    nc.sync.dma_start(out=out_t[tbase + j], in_=xb[c][:, j, :])
                tbase += k

    nc.finalize()
    return nc


@functools.cache
def _get_nc():
    return build_kernel()


def kernel(x, M, S, _trace=False, _trace_kwargs=None):
    del M  # overwritten by the first Welford step in the reference
    x = np.ascontiguousarray(x, dtype=np.float32)
    S = np.ascontiguousarray(S, dtype=np.float32).reshape(1, F)
    nc = _get_nc()
    in_maps = [
        {"x": x[i * ROWS : (i + 1) * ROWS], "S": S} for i in range(NCORES)
    ]
    res = run_bass_kernel_spmd(
        nc,
        in_maps,
        core_ids=list(range(NCORES)),
        trace=_trace,
        **(_trace_kwargs or {}),
    )
    out = np.concatenate([res.results[i]["out"] for i in range(NCORES)], axis=0)
    if _trace:
        return out, res
    return out
